# revision 37
# baseline (speedup 1.0000x reference)
"""CategorySpecificLinear Trainium2 kernel.

out[t] = x[t] @ weight[category_id[t]] + bias[category_id[t]]

Strategy: expert-parallel over the 8 categories (C == n_cores == 8) with a
fixed device capacity of CAP=512 tokens per core. Host routes tokens by
category; the few tokens beyond 512 in an over-subscribed category (counts
are ~512 +/- 25 for T=4096 uniform tokens) are computed on the host during
the unshard step, so the NEFF shape is static.

All device traffic is fp16 (tolerance is 2e-2; fp16 in/out measures ~4e-4):
    xT  [D=1024, 512]  tokens of category c, transposed, zero-padded
    w   [D, O]         weight[c]
    out [512, O]       fp16; bias (+ fp32 cast) is folded into the host
                       scatter -- a vectorized add during unsharding.

Compute is x-stationary: psum[m,n] (+)= x[k,m].T @ w[k,n] over k, with
m = 4 token-tiles of 128 and n = 2 O-halves of 512 -- exactly the 8 fp32
PSUM banks, accumulating in lockstep k-outer so each k-step is gated only
on slice k's DMA and just the last 8 matmuls run after slice 7 lands.
The last k-step interleaves each group's psum->fp16 cast (DVE for n=0,
ACT for n=1 in parallel) and store with the remaining matmuls.

Loads stream k-major on all three queues (w-lo on Sync, w-hi on Scalar --
HWDGE handles the 1 KB @ 2 KB-stride halves; x on GpSimd, contiguous
only, since strided patterns on SWDGE peg the Q7 with software descriptor
emission). A ~3.9 us dummy-matmul warm-up bridges the PE from engine-free
(~6.6 us) to first-slice-landed (~10.5 us) with no idle gap, so the HAM
clock-gate is at 8/8 for the whole real stream (measured: 64 MMs in
14.0 us, zero stalls; a gap re-throttles to 1.2 GHz for ~2 us).

Per-core HBM traffic ~4.2 MB; measured 32.4 us NEFF exec vs 43.7 baseline
(~6.5 us fixed framework preamble + ~3 us teardown barriers included).
"""

import contextlib
import ctypes
import os
import sys
import types

import numpy as np

sys.path.insert(0, "/opt/trn_rl_repo")


def _ensure_ntff_hook():
    """Provide antenv.axon_hooks if the image lacks it.

    concourse.bass_utils imports antenv.axon_hooks.get_axon_ntff_profile_hook
    when trace=True under axon; some agent images don't ship that module, in
    which case the boot's NTFF hook registration silently degrades and the
    import in bass_utils crashes. Recreate the slim ctypes hook here
    (mirrors trn_agent_boot.trn_boot._ntff_profile_via_ctypes).
    """
    try:
        import antenv.axon_hooks  # noqa: F401

        return
    except ImportError:
        pass

    so_path = "/opt/axon/libaxon_pjrt.so"
    hook = None
    if os.path.exists(so_path):
        lib = ctypes.CDLL(so_path)
        if hasattr(lib, "axon_start_nrt_profile"):
            lib.axon_start_nrt_profile.argtypes = [
                ctypes.POINTER(ctypes.c_int64),
                ctypes.c_size_t,
            ]
            lib.axon_start_nrt_profile.restype = ctypes.c_int64
            lib.axon_stop_nrt_profile.argtypes = [ctypes.c_char_p]
            lib.axon_stop_nrt_profile.restype = ctypes.c_int64

            @contextlib.contextmanager
            def hook(output_dir, device_ids):
                import jax

                jax.devices()
                if device_ids:
                    ids = (ctypes.c_int64 * len(device_ids))(*device_ids)
                    rc = lib.axon_start_nrt_profile(ids, len(device_ids))
                else:
                    rc = lib.axon_start_nrt_profile(None, 0)
                if rc != 0:
                    raise RuntimeError(f"axon_start_nrt_profile rc={rc}")
                try:
                    yield
                finally:
                    n = lib.axon_stop_nrt_profile(str(output_dir).encode())
                    if n <= 0:
                        print(
                            f"ntff profile: rc={n} writing {output_dir}",
                            file=sys.stderr,
                        )

    mod = types.ModuleType("antenv.axon_hooks")
    _state = {"hook": hook}
    mod.set_axon_ntff_profile_hook = lambda h: _state.__setitem__("hook", h)
    mod.get_axon_ntff_profile_hook = lambda: _state["hook"]
    sys.modules["antenv.axon_hooks"] = mod
    try:
        import antenv

        antenv.axon_hooks = mod
    except ImportError:
        pass


_ensure_ntff_hook()

import concourse.bass as bass
import concourse.bacc as bacc_mod
import concourse.mybir as mybir
import concourse.tile as tile
from concourse.bass import ts
from concourse.bass_utils import run_bass_kernel_spmd

N_CORES = 8
P = 128
CAP = 512  # device tokens per core
D = 1024
O = 1024
KO = D // P  # 8 contraction slices
MO = CAP // P  # 4 token tiles
NT = 512  # O-half (one fp32 PSUM bank)
NO = O // NT  # 2

_nc_cache = {}
LAST_RESULTS = None  # BassKernelResults of the most recent run (for test.py)


def _build_nc():
    f16 = mybir.dt.float16
    f32 = mybir.dt.float32

    nc = bacc_mod.Bacc()
    # natural layouts: each k-slice load is a CONTIGUOUS (or half-row-
    # strided) DRAM block. (A host-side partition-major repack was tried
    # and scatters every piece into 1 KB lines at 8 KB stride -- queue
    # rates drop ~2x and SWDGE descriptor emission pegs the Q7.)
    xT = nc.dram_tensor("xT", [D, CAP], f16, kind="ExternalInput")
    w = nc.dram_tensor("w", [D, O], f16, kind="ExternalInput")
    out = nc.dram_tensor("out", [CAP, O], f16, kind="ExternalOutput")

    xT_t = xT[:, :].rearrange("(ko p) t -> p ko t", p=P)
    w_t = w[:, :].rearrange("(ko p) o -> p ko o", p=P)

    with tile.TileContext(nc) as tc:
        with (
            tc.tile_pool(name="resident", bufs=1) as rpool,
            tc.tile_pool(name="psum", bufs=8, space="PSUM") as psum_pool,
            tc.tile_pool(name="obuf", bufs=8) as opool,
        ):
            # HAM warm-up, BRIDGING into the real stream with no idle gap:
            # HAM un-throttles only after ~3.4 us of sustained PE busy, and
            # an idle gap before the first real MM restarts that clock (a
            # 1.7 us gap measured ~2 us of half-rate real MMs). 36 N=128
            # MMs cover engine-free (~6.6 us) to first-slice (~10.5 us).
            # The warm psum tile is the first allocation of the 8-buf "ps"
            # ring; its bank is recycled for the last psum group (warm-up
            # is long done by that group's first MM).
            warm_sb = rpool.tile([P, P], f16, tag="warm")
            nc.vector.memset(warm_sb[:], 0.0)
            warm_ps = psum_pool.tile([P, NT], f32, tag="ps", name="warm_ps")
            for _ in range(36):
                nc.tensor.matmul(
                    warm_ps[:, :P],
                    lhsT=warm_sb[:],
                    rhs=warm_sb[:],
                    start=True,
                    stop=True,
                )

            # Loads k-major on all THREE queues, so every queue contributes
            # to every k-slice (~1.3 us/slice vs the PE's 1.75 us/step
            # consumption). Queue roles: the two HWDGE queues take the
            # STRIDED w halves (1 KB lines at 2 KB stride -- RTL descriptor
            # gen handles the stride, and Sync's earlier first-byte carries
            # the PE-gating w-lo); SWDGE (GpSimd) takes only the CONTIGUOUS
            # x slices -- strided patterns on SWDGE peg the Q7 with
            # software descriptor emission (measured ~1 us/slice slip).
            # x[0] is split into two 64 KB half-tiles so the m0/m1 matmuls
            # start ~0.8 us before the full slice would have landed (x0 is
            # SWDGE's first piece and otherwise gates the whole stream).
            x0a = rpool.tile([P, 2 * P], f16, tag="x0a")
            nc.gpsimd.dma_start(x0a[:], xT_t[:, 0, : 2 * P])
            x0b = rpool.tile([P, 2 * P], f16, tag="x0b")
            nc.gpsimd.dma_start(x0b[:], xT_t[:, 0, 2 * P :])
            x_sb, w_sb = [None], []
            for k in range(KO):
                if k > 0:
                    xt = rpool.tile([P, CAP], f16, tag=f"x{k}")
                    nc.gpsimd.dma_start(xt[:], xT_t[:, k, :])
                    x_sb.append(xt)
                wt = rpool.tile([P, O], f16, tag=f"w{k}")
                nc.sync.dma_start(wt[:, :NT], w_t[:, k, :NT])
                nc.scalar.dma_start(wt[:, NT:], w_t[:, k, NT:])
                w_sb.append(wt)

            def x_ap(k, m):
                if k == 0:
                    half = x0a if m < 2 else x0b
                    return half[:, ts(m % 2, P)]
                return x_sb[k][:, ts(m, P)]

            def w_ap(k, n):
                return w_sb[k][:, ts(n, NT)]

            pss = {
                (m, n): psum_pool.tile([P, NT], f32, tag="ps", name=f"ps{m}_{n}")
                for m in range(MO)
                for n in range(NO)
            }

            # Phase 1 k=0..4: every k-step gated only on slice k's DMA;
            # 40 MMs end (~19.8 us) right as the LAST slice lands (~19.7),
            # so the per-m tails below never wait on DMA. n-outer within k
            # so n=0 matmuls never wait for the w-hi piece.
            PH1 = 5
            # k=0 runs m-pair-major (m0/m1 over both n, then m2/m3) so the
            # first 4 matmuls need only the x0a half-tile.
            for m2 in range(0, MO, 2):
                for n in range(NO):
                    for m in (m2, m2 + 1):
                        nc.tensor.matmul(
                            pss[(m, n)][:],
                            lhsT=x_ap(0, m),
                            rhs=w_ap(0, n),
                            start=True,
                            stop=False,
                        )
            for k in range(1, PH1):
                for n in range(NO):
                    for m in range(MO):
                        nc.tensor.matmul(
                            pss[(m, n)][:],
                            lhsT=x_ap(k, m),
                            rhs=w_ap(k, n),
                            start=False,
                            stop=False,
                        )
            # Per-m tails k=5..7: group m completes ~1.3 us after m-1, so
            # the casts (DVE n=0 / ACT n=1, parallel psum banks) and the
            # half-row stores (separate HWDGE queues) of m pipeline UNDER
            # the remaining matmuls; only m3's cast+store trail the PE.
            # (With an earlier boundary the tails chain behind the k=7
            # arrival -- v4 measured 3 us worse with boundary k=4.)
            for m in range(MO):
                for k in range(PH1, KO):
                    for n in range(NO):
                        nc.tensor.matmul(
                            pss[(m, n)][:],
                            lhsT=x_ap(k, m),
                            rhs=w_ap(k, n),
                            start=False,
                            stop=(k == KO - 1),
                        )
                ot0 = opool.tile([P, NT], f16, tag="ot", name=f"ot{m}_0")
                nc.vector.tensor_copy(out=ot0[:], in_=pss[(m, 0)][:])
                nc.sync.dma_start(out[ts(m, P), :NT], ot0[:])
                ot1 = opool.tile([P, NT], f16, tag="ot", name=f"ot{m}_1")
                nc.scalar.activation(
                    ot1[:],
                    pss[(m, 1)][:],
                    mybir.ActivationFunctionType.Copy,
                )
                nc.scalar.dma_start(out[ts(m, P), NT:], ot1[:])
    nc.finalize()
    return nc


def kernel(x, category_id, weight, bias):
    global LAST_RESULTS
    x = np.asarray(x)
    category_id = np.asarray(category_id)
    weight = np.ascontiguousarray(np.asarray(weight), dtype=np.float32)
    bias = np.ascontiguousarray(np.asarray(bias), dtype=np.float32)

    orig_shape = x.shape
    d = orig_shape[-1]
    C, _, o = weight.shape
    assert C == N_CORES and d == D and o == O

    T = int(np.prod(orig_shape[:-1]))
    x_flat = np.ascontiguousarray(x.reshape(T, D), dtype=np.float32)
    cid = category_id.reshape(T).astype(np.int64)

    idx_per_c = [np.flatnonzero(cid == c) for c in range(C)]
    dev_idx = [ix[:CAP] for ix in idx_per_c]
    over_idx = [ix[CAP:] for ix in idx_per_c]

    if "nc" not in _nc_cache:
        _nc_cache["nc"] = _build_nc()
    nc = _nc_cache["nc"]

    in_maps = []
    for c in range(C):
        xcT = np.zeros((D, CAP), dtype=np.float16)
        n = len(dev_idx[c])
        xcT[:, :n] = x_flat[dev_idx[c]].astype(np.float16).T
        in_maps.append({"xT": xcT, "w": weight[c].astype(np.float16)})

    res = run_bass_kernel_spmd(nc, in_maps, list(range(N_CORES)))
    LAST_RESULTS = res

    out_flat = np.empty((T, O), dtype=np.float32)
    for c in range(C):
        n = len(dev_idx[c])
        out_flat[dev_idx[c]] = res.results[c]["out"][:n].astype(np.float32) + bias[c]
        if len(over_idx[c]):
            # capacity overflow (counts are ~512±25; a handful of tokens):
            # exact fp32 on host as part of the unshard/scatter step
            out_flat[over_idx[c]] = x_flat[over_idx[c]] @ weight[c] + bias[c]
    return out_flat.reshape(*orig_shape[:-1], O)


# revision 38
# speedup vs baseline: 1.0075x; 1.0075x over previous
"""CategorySpecificLinear Trainium2 kernel.

out[t] = x[t] @ weight[category_id[t]] + bias[category_id[t]]

Strategy: expert-parallel over the 8 categories (C == n_cores == 8) with a
fixed device capacity of CAP=512 tokens per core. Host routes tokens by
category; the few tokens beyond 512 in an over-subscribed category (counts
are ~512 +/- 25 for T=4096 uniform tokens) are computed on the host during
the unshard step, so the NEFF shape is static.

All device traffic is fp16 (tolerance is 2e-2; fp16 in/out measures ~4e-4):
    xT  [D=1024, 512]  tokens of category c, transposed, zero-padded
    w   [D, O]         weight[c]
    out [512, O]       fp16; bias (+ fp32 cast) is folded into the host
                       scatter -- a vectorized add during unsharding.

Compute is x-stationary: psum[m,n] (+)= x[k,m].T @ w[k,n] over k, with
m = 4 token-tiles of 128 and n = 2 O-halves of 512 -- exactly the 8 fp32
PSUM banks, accumulating in lockstep k-outer so each k-step is gated only
on slice k's DMA and just the last 8 matmuls run after slice 7 lands.
The last k-step interleaves each group's psum->fp16 cast (DVE for n=0,
ACT for n=1 in parallel) and store with the remaining matmuls.

Loads stream k-major on all three queues (w-lo on Sync, w-hi on Scalar --
HWDGE handles the 1 KB @ 2 KB-stride halves; x on GpSimd, contiguous
only, since strided patterns on SWDGE peg the Q7 with software descriptor
emission). A ~3.9 us dummy-matmul warm-up bridges the PE from engine-free
(~6.6 us) to first-slice-landed (~10.5 us) with no idle gap, so the HAM
clock-gate is at 8/8 for the whole real stream (measured: 64 MMs in
14.0 us, zero stalls; a gap re-throttles to 1.2 GHz for ~2 us).

Per-core HBM traffic ~4.2 MB; measured 32.4 us NEFF exec vs 43.7 baseline
(~6.5 us fixed framework preamble + ~3 us teardown barriers included).
"""

import contextlib
import ctypes
import os
import sys
import types

import numpy as np

sys.path.insert(0, "/opt/trn_rl_repo")


def _ensure_ntff_hook():
    """Provide antenv.axon_hooks if the image lacks it.

    concourse.bass_utils imports antenv.axon_hooks.get_axon_ntff_profile_hook
    when trace=True under axon; some agent images don't ship that module, in
    which case the boot's NTFF hook registration silently degrades and the
    import in bass_utils crashes. Recreate the slim ctypes hook here
    (mirrors trn_agent_boot.trn_boot._ntff_profile_via_ctypes).
    """
    try:
        import antenv.axon_hooks  # noqa: F401

        return
    except ImportError:
        pass

    so_path = "/opt/axon/libaxon_pjrt.so"
    hook = None
    if os.path.exists(so_path):
        lib = ctypes.CDLL(so_path)
        if hasattr(lib, "axon_start_nrt_profile"):
            lib.axon_start_nrt_profile.argtypes = [
                ctypes.POINTER(ctypes.c_int64),
                ctypes.c_size_t,
            ]
            lib.axon_start_nrt_profile.restype = ctypes.c_int64
            lib.axon_stop_nrt_profile.argtypes = [ctypes.c_char_p]
            lib.axon_stop_nrt_profile.restype = ctypes.c_int64

            @contextlib.contextmanager
            def hook(output_dir, device_ids):
                import jax

                jax.devices()
                if device_ids:
                    ids = (ctypes.c_int64 * len(device_ids))(*device_ids)
                    rc = lib.axon_start_nrt_profile(ids, len(device_ids))
                else:
                    rc = lib.axon_start_nrt_profile(None, 0)
                if rc != 0:
                    raise RuntimeError(f"axon_start_nrt_profile rc={rc}")
                try:
                    yield
                finally:
                    n = lib.axon_stop_nrt_profile(str(output_dir).encode())
                    if n <= 0:
                        print(
                            f"ntff profile: rc={n} writing {output_dir}",
                            file=sys.stderr,
                        )

    mod = types.ModuleType("antenv.axon_hooks")
    _state = {"hook": hook}
    mod.set_axon_ntff_profile_hook = lambda h: _state.__setitem__("hook", h)
    mod.get_axon_ntff_profile_hook = lambda: _state["hook"]
    sys.modules["antenv.axon_hooks"] = mod
    try:
        import antenv

        antenv.axon_hooks = mod
    except ImportError:
        pass


_ensure_ntff_hook()

import concourse.bass as bass
import concourse.bacc as bacc_mod
import concourse.mybir as mybir
import concourse.tile as tile
from concourse.bass import ts
from concourse.bass_utils import run_bass_kernel_spmd

N_CORES = 8
P = 128
CAP = 512  # device tokens per core
D = 1024
O = 1024
KO = D // P  # 8 contraction slices
MO = CAP // P  # 4 token tiles
NT = 512  # O-half (one fp32 PSUM bank)
NO = O // NT  # 2

_nc_cache = {}
LAST_RESULTS = None  # BassKernelResults of the most recent run (for test.py)


def _build_nc():
    f16 = mybir.dt.float16
    f32 = mybir.dt.float32

    nc = bacc_mod.Bacc()
    # natural layouts: each k-slice load is a CONTIGUOUS (or half-row-
    # strided) DRAM block. (A host-side partition-major repack was tried
    # and scatters every piece into 1 KB lines at 8 KB stride -- queue
    # rates drop ~2x and SWDGE descriptor emission pegs the Q7.)
    xT = nc.dram_tensor("xT", [D, CAP], f16, kind="ExternalInput")
    w = nc.dram_tensor("w", [D, O], f16, kind="ExternalInput")
    out = nc.dram_tensor("out", [CAP, O], f16, kind="ExternalOutput")

    xT_t = xT[:, :].rearrange("(ko p) t -> p ko t", p=P)
    w_t = w[:, :].rearrange("(ko p) o -> p ko o", p=P)

    with tile.TileContext(nc) as tc:
        with (
            tc.tile_pool(name="resident", bufs=1) as rpool,
            tc.tile_pool(name="psum", bufs=8, space="PSUM") as psum_pool,
            tc.tile_pool(name="obuf", bufs=8) as opool,
        ):
            # HAM warm-up, BRIDGING into the real stream with no idle gap:
            # HAM un-throttles only after ~3.4 us of sustained PE busy, and
            # an idle gap before the first real MM restarts that clock (a
            # 1.7 us gap measured ~2 us of half-rate real MMs). 36 N=128
            # MMs cover engine-free (~6.6 us) to first-slice (~10.5 us).
            # The warm psum tile is the first allocation of the 8-buf "ps"
            # ring; its bank is recycled for the last psum group (warm-up
            # is long done by that group's first MM).
            warm_sb = rpool.tile([P, P], f16, tag="warm")
            nc.vector.memset(warm_sb[:], 0.0)
            warm_ps = psum_pool.tile([P, NT], f32, tag="ps", name="warm_ps")
            for _ in range(36):
                nc.tensor.matmul(
                    warm_ps[:, :P],
                    lhsT=warm_sb[:],
                    rhs=warm_sb[:],
                    start=True,
                    stop=True,
                )

            # Loads k-major on all THREE queues, so every queue contributes
            # to every k-slice (~1.3 us/slice vs the PE's 1.75 us/step
            # consumption). Queue roles: the two HWDGE queues take the
            # STRIDED w halves (1 KB lines at 2 KB stride -- RTL descriptor
            # gen handles the stride, and Sync's earlier first-byte carries
            # the PE-gating w-lo); SWDGE (GpSimd) takes only the CONTIGUOUS
            # x slices -- strided patterns on SWDGE peg the Q7 with
            # software descriptor emission (measured ~1 us/slice slip).
            x_sb, w_sb = [], []
            for k in range(KO):
                xt = rpool.tile([P, CAP], f16, tag=f"x{k}")
                nc.gpsimd.dma_start(xt[:], xT_t[:, k, :])
                x_sb.append(xt)
                wt = rpool.tile([P, O], f16, tag=f"w{k}")
                nc.sync.dma_start(wt[:, :NT], w_t[:, k, :NT])
                nc.scalar.dma_start(wt[:, NT:], w_t[:, k, NT:])
                w_sb.append(wt)

            def x_ap(k, m):
                return x_sb[k][:, ts(m, P)]

            def w_ap(k, n):
                return w_sb[k][:, ts(n, NT)]

            pss = {
                (m, n): psum_pool.tile([P, NT], f32, tag="ps", name=f"ps{m}_{n}")
                for m in range(MO)
                for n in range(NO)
            }

            # Phase 1 k=0..4: every k-step gated only on slice k's DMA;
            # 40 MMs end (~19.8 us) right as the LAST slice lands (~19.7),
            # so the per-m tails below never wait on DMA. n-outer within k
            # so n=0 matmuls never wait for the w-hi piece.
            PH1 = 5
            for k in range(PH1):
                for n in range(NO):
                    for m in range(MO):
                        nc.tensor.matmul(
                            pss[(m, n)][:],
                            lhsT=x_ap(k, m),
                            rhs=w_ap(k, n),
                            start=(k == 0),
                            stop=False,
                        )
            # Per-m tails k=5..7: group m completes ~1.3 us after m-1, so
            # the casts (DVE n=0 / ACT n=1, parallel psum banks) and the
            # half-row stores (separate HWDGE queues) of m pipeline UNDER
            # the remaining matmuls; only m3's cast+store trail the PE.
            # (With an earlier boundary the tails chain behind the k=7
            # arrival -- v4 measured 3 us worse with boundary k=4.)
            for m in range(MO):
                for k in range(PH1, KO):
                    for n in range(NO):
                        nc.tensor.matmul(
                            pss[(m, n)][:],
                            lhsT=x_ap(k, m),
                            rhs=w_ap(k, n),
                            start=False,
                            stop=(k == KO - 1),
                        )
                ot0 = opool.tile([P, NT], f16, tag="ot", name=f"ot{m}_0")
                nc.vector.tensor_copy(out=ot0[:], in_=pss[(m, 0)][:])
                nc.sync.dma_start(out[ts(m, P), :NT], ot0[:])
                ot1 = opool.tile([P, NT], f16, tag="ot", name=f"ot{m}_1")
                nc.scalar.activation(
                    ot1[:],
                    pss[(m, 1)][:],
                    mybir.ActivationFunctionType.Copy,
                )
                nc.scalar.dma_start(out[ts(m, P), NT:], ot1[:])
    nc.finalize()
    return nc


def kernel(x, category_id, weight, bias):
    global LAST_RESULTS
    x = np.asarray(x)
    category_id = np.asarray(category_id)
    weight = np.ascontiguousarray(np.asarray(weight), dtype=np.float32)
    bias = np.ascontiguousarray(np.asarray(bias), dtype=np.float32)

    orig_shape = x.shape
    d = orig_shape[-1]
    C, _, o = weight.shape
    assert C == N_CORES and d == D and o == O

    T = int(np.prod(orig_shape[:-1]))
    x_flat = np.ascontiguousarray(x.reshape(T, D), dtype=np.float32)
    cid = category_id.reshape(T).astype(np.int64)

    idx_per_c = [np.flatnonzero(cid == c) for c in range(C)]
    dev_idx = [ix[:CAP] for ix in idx_per_c]
    over_idx = [ix[CAP:] for ix in idx_per_c]

    if "nc" not in _nc_cache:
        _nc_cache["nc"] = _build_nc()
    nc = _nc_cache["nc"]

    in_maps = []
    for c in range(C):
        xcT = np.zeros((D, CAP), dtype=np.float16)
        n = len(dev_idx[c])
        xcT[:, :n] = x_flat[dev_idx[c]].astype(np.float16).T
        in_maps.append({"xT": xcT, "w": weight[c].astype(np.float16)})

    res = run_bass_kernel_spmd(nc, in_maps, list(range(N_CORES)))
    LAST_RESULTS = res

    out_flat = np.empty((T, O), dtype=np.float32)
    for c in range(C):
        n = len(dev_idx[c])
        out_flat[dev_idx[c]] = res.results[c]["out"][:n].astype(np.float32) + bias[c]
        if len(over_idx[c]):
            # capacity overflow (counts are ~512±25; a handful of tokens):
            # exact fp32 on host as part of the unshard/scatter step
            out_flat[over_idx[c]] = x_flat[over_idx[c]] @ weight[c] + bias[c]
    return out_flat.reshape(*orig_shape[:-1], O)
